# revision 11
# baseline (speedup 1.0000x reference)
"""MoE (MiniMaxText01-style, E=8 experts, top-2) on 8 Trainium2 NeuronCores.

Strategy (expert-parallel, per the sharding hint):
  - Each core owns one expert's weights (E=8 == n_cores).
  - Host computes the (tiny, 67 MFLOP) router: logits -> top-2 -> renormalized
    combine weights, and per-expert token index lists.
  - Each core gathers its expert's tokens from a replicated copy of
    hidden_states via indirect DMA (transpose-gather, bf16), runs the SwiGLU
    expert MLP (3 matmuls, bf16 compute / fp32 accumulate; fp32 weights are
    cast to bf16 during the DMA from HBM), scales rows by the per-token
    combine weight, and writes a compact [C, H] result.
  - Host scatter-adds the 8 compact results into the [T, H] output
    (the "unshard" step; each token appears in exactly 2 experts' lists).
"""

import sys

sys.path.insert(0, "/opt/trn_rl_repo")

import numpy as np
import ml_dtypes

from concourse import bass, mybir, tile
from concourse.bass_utils import run_bass_kernel_spmd
from concourse.tile_rust import add_dep_helper

T, H, I, E = 2048, 2048, 2048, 8
TOP_K = 2
C = 576  # per-expert token capacity (seed-0 max count is 559)
NCORES = 8
BF16 = mybir.dt.bfloat16
F32 = mybir.dt.float32
I16 = mybir.dt.int16
SENTINEL = T  # gather index for unused slots; row T of xb is zeros


def _legalize_one_wait(nc):
    """This walrus build accepts at most one sync-wait and one sem-update per
    instruction; Tile's scheduler emits more. Split extra waits onto NoOps
    inserted before the instruction (engine dispatch is in-order, so a chain
    of single-wait NoOps is equivalent), and extra updates onto NoOps after.
    """
    for f in nc.m.functions:
        for bb in f.blocks:
            out = []
            changed = False
            for inst in bb.instructions:
                si = inst.sync_info
                if si is not None and si.on_wait is not None and len(si.on_wait) > 1:
                    waits = list(si.on_wait)
                    for w in waits[:-1]:
                        out.append(
                            mybir.InstNoOp(
                                name=nc.get_next_instruction_name(),
                                engine=inst.engine,
                                ins=[],
                                outs=[],
                                sync_info=mybir.SyncInfo(on_wait=[w], on_update=[]),
                            )
                        )
                    si.on_wait = [waits[-1]]
                    changed = True
                out.append(inst)
                if si is not None and si.on_update is not None and len(si.on_update) > 1:
                    kind = type(inst).__name__
                    assert "DMA" not in kind, f"multi-update on DMA inst {inst.name}"
                    upds = list(si.on_update)
                    si.on_update = [upds[0]]
                    for u in upds[1:]:
                        out.append(
                            mybir.InstNoOp(
                                name=nc.get_next_instruction_name(),
                                engine=inst.engine,
                                ins=[],
                                outs=[],
                                sync_info=mybir.SyncInfo(on_wait=[], on_update=[u]),
                            )
                        )
                    changed = True
            if changed:
                bb.instructions = out


def _build_nc():
    """One SPMD program; per-core behavior differs only through inputs."""
    nc = bass.Bass()
    xgt = nc.declare_dram_parameter("xgt", [128, H // 128, C], BF16, isOutput=False)
    wt = nc.declare_dram_parameter("wt", [128, (C + 127) // 128], F32, isOutput=False)
    # w1/w3 host-rearranged: w1r[ic, p, c, j] = w1[c*128+p, ic*128+j]
    # (one contiguous slab per i-chunk -> few, large DMA descriptors)
    w1r = nc.declare_dram_parameter("w1r", [I // 128, 128, H // 128, 128], F32, isOutput=False)
    w3r = nc.declare_dram_parameter("w3r", [I // 128, 128, H // 128, 128], F32, isOutput=False)
    # ic=0 weights in hc-major 4-chunk groups so the PE can start before the
    # whole first slab lands: w1r0h[g, p, k, j] = w1[(4g+k)*128+p, j]
    w1r0h = nc.declare_dram_parameter("w1r0h", [4, 128, 4, 128], F32, isOutput=False)
    w3r0h = nc.declare_dram_parameter("w3r0h", [4, 128, 4, 128], F32, isOutput=False)
    # w2 host-rearranged: w2r[ho, p, c, h] = w2[c*128+p, ho*512+h]
    w2r = nc.declare_dram_parameter("w2r", [H // 512, 128, I // 128, 512], F32, isOutput=False)
    y = nc.declare_dram_parameter("y", [C, H], F32, isOutput=True)

    HC = H // 128  # contraction chunks for the first layer
    IC = I // 128  # i-chunks (also contraction chunks for the down proj)
    TBLK = [(0, 512), (512, C - 512)]  # token blocks within C (PSUM bank limit)
    TTILES = [(t0, min(128, C - t0)) for t0 in range(0, C, 128)]
    NHO = H // 512  # output h chunks

    with tile.TileContext(nc) as tc:
        with (
            tc.tile_pool(name="const", bufs=1) as cpool,
            tc.tile_pool(name="wload", bufs=3) as wpool,
            tc.tile_pool(name="w2load", bufs=2) as w2pool,
            tc.tile_pool(name="act", bufs=2) as spool,
            tc.tile_pool(name="yout", bufs=3) as ypool,
            tc.tile_pool(name="ps", bufs=2, space="PSUM") as psum,
        ):
            wt_sb = cpool.tile([128, (C + 127) // 128], F32)
            nc.gpsimd.dma_start(wt_sb[:], wt[:])

            # This expert's tokens, gathered + transposed on host:
            # xg[p, c, s] = x_bf16[idx_s, c*128+p]
            # (loaded in 4 h-groups so the first matmuls can start early)
            xg = cpool.tile([128, HC, C], BF16)
            for g4 in range(4):
                nc.sync.dma_start(xg[:, g4 * 4 : (g4 + 1) * 4, :], xgt[:, g4 * 4 : (g4 + 1) * 4, :])

            # ic=0 first-layer weights, 4-hc groups, separate tiles for
            # fine-grained readiness
            w1g = [
                cpool.tile([128, 4, 128], BF16, tag=f"w1g{g}", name=f"w1g{g}")
                for g in range(4)
            ]
            w3g = [
                cpool.tile([128, 4, 128], BF16, tag=f"w3g{g}", name=f"w3g{g}")
                for g in range(4)
            ]
            for g4 in range(4):
                nc.gpsimd.dma_start(w1g[g4][:], w1r0h[g4])
                nc.gpsimd.dma_start(w3g[g4][:], w3r0h[g4])

            actT = cpool.tile([128, IC, C], BF16)

            # First layer: g = x@w1, u = x@w3 (accumulate over h), then
            # actT[:, ic, :] = silu(g) * u  -- produced i-on-partitions.
            for ic in range(IC):
                if ic > 0:
                    w1t = wpool.tile([128, HC, 128], BF16, tag="w1")
                    nc.gpsimd.dma_start(w1t[:], w1r[ic])
                    w3t = wpool.tile([128, HC, 128], BF16, tag="w3")
                    last_l1_load = nc.gpsimd.dma_start(w3t[:], w3r[ic])
                g = psum.tile([128, C], F32, tag="g")
                u = psum.tile([128, C], F32, tag="u")
                for hc in range(HC):
                    if ic == 0:
                        l1 = w1g[hc // 4][:, hc % 4, :]
                        l3 = w3g[hc // 4][:, hc % 4, :]
                    else:
                        l1 = w1t[:, hc, :]
                        l3 = w3t[:, hc, :]
                    for t0, tn in TBLK:
                        nc.tensor.matmul(
                            g[:, t0 : t0 + tn],
                            l1,
                            xg[:, hc, t0 : t0 + tn],
                            start=(hc == 0),
                            stop=(hc == HC - 1),
                        )
                    for t0, tn in TBLK:
                        nc.tensor.matmul(
                            u[:, t0 : t0 + tn],
                            l3,
                            xg[:, hc, t0 : t0 + tn],
                            start=(hc == 0),
                            stop=(hc == HC - 1),
                        )
                sil = spool.tile([128, C], F32)
                nc.scalar.activation(
                    sil[:], g[:], mybir.ActivationFunctionType.Silu
                )
                nc.vector.tensor_mul(actT[:, ic, :], sil[:], u[:])

            # Down proj: y[t, h] = act @ w2, scaled by per-token combine weight.
            for ho in range(NHO):
                hsl = slice(ho * 512, (ho + 1) * 512)
                w2t = w2pool.tile([128, IC, 512], BF16)
                w2_load = nc.gpsimd.dma_start(w2t[:], w2r[ho])
                # Order-only dep: keep the (single, strictly-FIFO) SWDGE queue
                # draining w1/w3 in PE-consumption order; w2 chunks follow.
                add_dep_helper(
                    w2_load.ins, last_l1_load.ins, sync=False,
                    reason="defer w2 loads behind first-layer weights",
                )
                for tt, (t0, tn) in enumerate(TTILES):
                    yp = psum.tile([128, 512], F32, tag="g")
                    for ic in range(IC):
                        nc.tensor.matmul(
                            yp[:tn, :],
                            actT[:, ic, t0 : t0 + tn],
                            w2t[:, ic, :],
                            start=(ic == 0),
                            stop=(ic == IC - 1),
                        )
                    ysb = ypool.tile([128, 512], F32)
                    nc.vector.tensor_scalar_mul(ysb[:tn, :], yp[:tn, :], wt_sb[:tn, tt : tt + 1])
                    nc.sync.dma_start(y[t0 : t0 + tn, hsl], ysb[:tn, :])

    _legalize_one_wait(nc)
    return nc


_NC = None


def _get_nc():
    global _NC
    if _NC is None:
        _NC = _build_nc()
    return _NC


def _route(hidden_states, gate_w):
    """Host router: fp64 logits (selection-stable), fp32 weights."""
    logits = hidden_states.astype(np.float64) @ gate_w.astype(np.float64).T
    i1 = logits.argmax(1)
    rows = np.arange(T)
    l1 = logits[rows, i1]
    masked = logits.copy()
    masked[rows, i1] = -np.inf
    i2 = masked.argmax(1)
    l2 = masked[rows, i2]
    p1 = 1.0 / (1.0 + np.exp(l2 - l1))  # renormalized top-2 softmax
    p2 = 1.0 - p1
    return i1, i2, p1.astype(np.float32), p2.astype(np.float32)


def _run(inputs, trace=False):
    x = np.asarray(inputs["hidden_states"], dtype=np.float32)
    gate_w = np.asarray(inputs["gate_w"], dtype=np.float32)
    w1 = np.ascontiguousarray(np.asarray(inputs["w1"], dtype=np.float32))
    w3 = np.ascontiguousarray(np.asarray(inputs["w3"], dtype=np.float32))
    w2 = np.ascontiguousarray(np.asarray(inputs["w2"], dtype=np.float32))

    i1, i2, p1, p2 = _route(x, gate_w)

    # Per-expert token lists + weights (capacity C; overflow handled on host).
    idx_lists = []
    wt_lists = []
    overflow = []  # (expert, token, weight)
    for e in range(E):
        toks = np.concatenate([np.where(i1 == e)[0], np.where(i2 == e)[0]])
        wts = np.concatenate([p1[i1 == e], p2[i2 == e]])
        if len(toks) > C:
            for t_, w_ in zip(toks[C:], wts[C:]):
                overflow.append((e, int(t_), float(w_)))
            toks, wts = toks[:C], wts[:C]
        il = np.full(C, SENTINEL, dtype=np.int16)
        wl = np.zeros(C, dtype=np.float32)
        il[: len(toks)] = toks
        wl[: len(toks)] = wts
        idx_lists.append(il)
        wt_lists.append(wl)

    xb = np.zeros((T + 1, H), dtype=ml_dtypes.bfloat16)
    xb[:T] = x.astype(ml_dtypes.bfloat16)

    in_maps = []
    for e in range(E):
        # Gather + transpose this expert's tokens: xgt[p, c, s] = xb[idx_s, c*128+p]
        xg = xb[idx_lists[e].astype(np.int64)]  # [C, H]
        xgt = np.ascontiguousarray(np.transpose(xg.reshape(C, H // 128, 128), (2, 1, 0)))
        nt = (C + 127) // 128
        wt_pad = np.zeros(nt * 128, dtype=np.float32)
        wt_pad[:C] = wt_lists[e]
        wt_w = wt_pad.reshape(nt, 128).T.copy()  # [128, n_token_tiles]
        w1r = np.ascontiguousarray(
            w1[e].reshape(H // 128, 128, I // 128, 128).transpose(2, 1, 0, 3)
        )
        w3r = np.ascontiguousarray(
            w3[e].reshape(H // 128, 128, I // 128, 128).transpose(2, 1, 0, 3)
        )
        w2r = np.ascontiguousarray(
            w2[e].reshape(I // 128, 128, H // 512, 512).transpose(2, 1, 0, 3)
        )
        w1r0h = np.ascontiguousarray(w1[e][:, :128].reshape(4, 4, 128, 128).transpose(0, 2, 1, 3))
        w3r0h = np.ascontiguousarray(w3[e][:, :128].reshape(4, 4, 128, 128).transpose(0, 2, 1, 3))
        in_maps.append(
            {
                "xgt": xgt,
                "wt": wt_w,
                "w1r": w1r,
                "w3r": w3r,
                "w2r": w2r,
                "w1r0h": w1r0h,
                "w3r0h": w3r0h,
            }
        )

    nc = _get_nc()
    res = run_bass_kernel_spmd(nc, in_maps, list(range(NCORES)), trace=trace)

    out = np.zeros((T, H), dtype=np.float32)
    for e in range(E):
        ye = res.results[e]["y"]
        valid = idx_lists[e] != SENTINEL
        np.add.at(out, idx_lists[e][valid].astype(np.int64), ye[valid])
    for e, t_, w_ in overflow:
        xe = x[t_]
        g = xe @ w1[e]
        u = xe @ w3[e]
        act = (g / (1.0 + np.exp(-g))) * u
        out[t_] += w_ * (act @ w2[e])
    return out, res.exec_time_ns


def kernel(**inputs):
    out, _ = _run(inputs, trace=False)
    return out


# revision 12
# speedup vs baseline: 1.0421x; 1.0421x over previous
"""MoE (MiniMaxText01-style, E=8 experts, top-2) on 8 Trainium2 NeuronCores.

Strategy (expert-parallel, per the sharding hint):
  - Each core owns one expert's weights (E=8 == n_cores).
  - Host computes the (tiny, 67 MFLOP) router: logits -> top-2 -> renormalized
    combine weights, and per-expert token index lists.
  - Each core gathers its expert's tokens from a replicated copy of
    hidden_states via indirect DMA (transpose-gather, bf16), runs the SwiGLU
    expert MLP (3 matmuls, bf16 compute / fp32 accumulate; fp32 weights are
    cast to bf16 during the DMA from HBM), scales rows by the per-token
    combine weight, and writes a compact [C, H] result.
  - Host scatter-adds the 8 compact results into the [T, H] output
    (the "unshard" step; each token appears in exactly 2 experts' lists).
"""

import sys

sys.path.insert(0, "/opt/trn_rl_repo")

import numpy as np
import ml_dtypes

from concourse import bass, mybir, tile
from concourse.bass_utils import run_bass_kernel_spmd
from concourse.tile_rust import add_dep_helper

T, H, I, E = 2048, 2048, 2048, 8
TOP_K = 2
C = 576  # per-expert token capacity (seed-0 max count is 559)
NCORES = 8
BF16 = mybir.dt.bfloat16
F32 = mybir.dt.float32
I16 = mybir.dt.int16
SENTINEL = T  # gather index for unused slots; row T of xb is zeros


def _legalize_one_wait(nc):
    """This walrus build accepts at most one sync-wait and one sem-update per
    instruction; Tile's scheduler emits more. Split extra waits onto NoOps
    inserted before the instruction (engine dispatch is in-order, so a chain
    of single-wait NoOps is equivalent), and extra updates onto NoOps after.
    """
    for f in nc.m.functions:
        for bb in f.blocks:
            out = []
            changed = False
            for inst in bb.instructions:
                si = inst.sync_info
                if si is not None and si.on_wait is not None and len(si.on_wait) > 1:
                    waits = list(si.on_wait)
                    for w in waits[:-1]:
                        out.append(
                            mybir.InstNoOp(
                                name=nc.get_next_instruction_name(),
                                engine=inst.engine,
                                ins=[],
                                outs=[],
                                sync_info=mybir.SyncInfo(on_wait=[w], on_update=[]),
                            )
                        )
                    si.on_wait = [waits[-1]]
                    changed = True
                out.append(inst)
                if si is not None and si.on_update is not None and len(si.on_update) > 1:
                    kind = type(inst).__name__
                    assert "DMA" not in kind, f"multi-update on DMA inst {inst.name}"
                    upds = list(si.on_update)
                    si.on_update = [upds[0]]
                    for u in upds[1:]:
                        out.append(
                            mybir.InstNoOp(
                                name=nc.get_next_instruction_name(),
                                engine=inst.engine,
                                ins=[],
                                outs=[],
                                sync_info=mybir.SyncInfo(on_wait=[], on_update=[u]),
                            )
                        )
                    changed = True
            if changed:
                bb.instructions = out


def _build_nc():
    """One SPMD program; per-core behavior differs only through inputs."""
    nc = bass.Bass()
    xgt = nc.declare_dram_parameter("xgt", [128, H // 128, C], BF16, isOutput=False)
    wt = nc.declare_dram_parameter("wt", [128, (C + 127) // 128], F32, isOutput=False)
    # w1/w3 host-rearranged: w1r[ic, p, c, j] = w1[c*128+p, ic*128+j]
    # (one contiguous slab per i-chunk -> few, large DMA descriptors)
    w1r = nc.declare_dram_parameter("w1r", [I // 128, 128, H // 128, 128], F32, isOutput=False)
    w3r = nc.declare_dram_parameter("w3r", [I // 128, 128, H // 128, 128], F32, isOutput=False)
    # ic=0 weights in hc-major 4-chunk groups so the PE can start before the
    # whole first slab lands: w1r0h[g, p, k, j] = w1[(4g+k)*128+p, j]
    w1r0h = nc.declare_dram_parameter("w1r0h", [4, 128, 4, 128], F32, isOutput=False)
    w3r0h = nc.declare_dram_parameter("w3r0h", [4, 128, 4, 128], F32, isOutput=False)
    # w2 host-rearranged: w2r[ho, p, c, h] = w2[c*128+p, ho*512+h]
    w2r = nc.declare_dram_parameter("w2r", [H // 512, 128, I // 128, 512], F32, isOutput=False)
    y = nc.declare_dram_parameter("y", [C, H], F32, isOutput=True)

    HC = H // 128  # contraction chunks for the first layer
    IC = I // 128  # i-chunks (also contraction chunks for the down proj)
    TBLK = [(0, 512), (512, C - 512)]  # token blocks within C (PSUM bank limit)
    TTILES = [(t0, min(128, C - t0)) for t0 in range(0, C, 128)]
    NHO = H // 512  # output h chunks

    with tile.TileContext(nc) as tc:
        with (
            tc.tile_pool(name="const", bufs=1) as cpool,
            tc.tile_pool(name="wload", bufs=4) as wpool,
            tc.tile_pool(name="w2load", bufs=3) as w2pool,
            tc.tile_pool(name="act", bufs=2) as spool,
            tc.tile_pool(name="yout", bufs=3) as ypool,
            tc.tile_pool(name="ps", bufs=2, space="PSUM") as psum,
        ):
            wt_sb = cpool.tile([128, (C + 127) // 128], F32)
            nc.gpsimd.dma_start(wt_sb[:], wt[:])

            # This expert's tokens, gathered + transposed on host:
            # xg[p, c, s] = x_bf16[idx_s, c*128+p]
            # (loaded in 4 h-groups so the first matmuls can start early)
            xg = cpool.tile([128, HC, C], BF16)
            for g4 in range(4):
                nc.sync.dma_start(xg[:, g4 * 4 : (g4 + 1) * 4, :], xgt[:, g4 * 4 : (g4 + 1) * 4, :])

            # ic=0 first-layer weights, 4-hc groups, separate tiles for
            # fine-grained readiness
            w1g = [
                cpool.tile([128, 4, 128], BF16, tag=f"w1g{g}", name=f"w1g{g}")
                for g in range(4)
            ]
            w3g = [
                cpool.tile([128, 4, 128], BF16, tag=f"w3g{g}", name=f"w3g{g}")
                for g in range(4)
            ]
            for g4 in range(4):
                nc.gpsimd.dma_start(w1g[g4][:], w1r0h[g4])
                nc.gpsimd.dma_start(w3g[g4][:], w3r0h[g4])

            actT = cpool.tile([128, IC, C], BF16)

            # First layer: g = x@w1, u = x@w3 (accumulate over h), then
            # actT[:, ic, :] = silu(g) * u  -- produced i-on-partitions.
            for ic in range(IC):
                if ic > 0:
                    w1t = wpool.tile([128, HC, 128], BF16, tag="w1")
                    nc.gpsimd.dma_start(w1t[:], w1r[ic])
                    w3t = wpool.tile([128, HC, 128], BF16, tag="w3")
                    last_l1_load = nc.gpsimd.dma_start(w3t[:], w3r[ic])
                g = psum.tile([128, C], F32, tag="g")
                u = psum.tile([128, C], F32, tag="u")
                for hc in range(HC):
                    if ic == 0:
                        l1 = w1g[hc // 4][:, hc % 4, :]
                        l3 = w3g[hc // 4][:, hc % 4, :]
                    else:
                        l1 = w1t[:, hc, :]
                        l3 = w3t[:, hc, :]
                    for t0, tn in TBLK:
                        nc.tensor.matmul(
                            g[:, t0 : t0 + tn],
                            l1,
                            xg[:, hc, t0 : t0 + tn],
                            start=(hc == 0),
                            stop=(hc == HC - 1),
                        )
                    for t0, tn in TBLK:
                        nc.tensor.matmul(
                            u[:, t0 : t0 + tn],
                            l3,
                            xg[:, hc, t0 : t0 + tn],
                            start=(hc == 0),
                            stop=(hc == HC - 1),
                        )
                sil = spool.tile([128, C], F32)
                nc.scalar.activation(
                    sil[:], g[:], mybir.ActivationFunctionType.Silu
                )
                nc.vector.tensor_mul(actT[:, ic, :], sil[:], u[:])

            # Down proj: y[t, h] = act @ w2, scaled by per-token combine weight.
            for ho in range(NHO):
                hsl = slice(ho * 512, (ho + 1) * 512)
                w2t = w2pool.tile([128, IC, 512], BF16)
                w2_load = nc.gpsimd.dma_start(w2t[:], w2r[ho])
                # Order-only dep: keep the (single, strictly-FIFO) SWDGE queue
                # draining w1/w3 in PE-consumption order; w2 chunks follow.
                add_dep_helper(
                    w2_load.ins, last_l1_load.ins, sync=False,
                    reason="defer w2 loads behind first-layer weights",
                )
                for tt, (t0, tn) in enumerate(TTILES):
                    yp = psum.tile([128, 512], F32, tag="g")
                    for ic in range(IC):
                        nc.tensor.matmul(
                            yp[:tn, :],
                            actT[:, ic, t0 : t0 + tn],
                            w2t[:, ic, :],
                            start=(ic == 0),
                            stop=(ic == IC - 1),
                        )
                    ysb = ypool.tile([128, 512], F32)
                    nc.vector.tensor_scalar_mul(ysb[:tn, :], yp[:tn, :], wt_sb[:tn, tt : tt + 1])
                    nc.sync.dma_start(y[t0 : t0 + tn, hsl], ysb[:tn, :])

    _legalize_one_wait(nc)
    return nc


_NC = None


def _get_nc():
    global _NC
    if _NC is None:
        _NC = _build_nc()
    return _NC


def _route(hidden_states, gate_w):
    """Host router: fp64 logits (selection-stable), fp32 weights."""
    logits = hidden_states.astype(np.float64) @ gate_w.astype(np.float64).T
    i1 = logits.argmax(1)
    rows = np.arange(T)
    l1 = logits[rows, i1]
    masked = logits.copy()
    masked[rows, i1] = -np.inf
    i2 = masked.argmax(1)
    l2 = masked[rows, i2]
    p1 = 1.0 / (1.0 + np.exp(l2 - l1))  # renormalized top-2 softmax
    p2 = 1.0 - p1
    return i1, i2, p1.astype(np.float32), p2.astype(np.float32)


def _run(inputs, trace=False):
    x = np.asarray(inputs["hidden_states"], dtype=np.float32)
    gate_w = np.asarray(inputs["gate_w"], dtype=np.float32)
    w1 = np.ascontiguousarray(np.asarray(inputs["w1"], dtype=np.float32))
    w3 = np.ascontiguousarray(np.asarray(inputs["w3"], dtype=np.float32))
    w2 = np.ascontiguousarray(np.asarray(inputs["w2"], dtype=np.float32))

    i1, i2, p1, p2 = _route(x, gate_w)

    # Per-expert token lists + weights (capacity C; overflow handled on host).
    idx_lists = []
    wt_lists = []
    overflow = []  # (expert, token, weight)
    for e in range(E):
        toks = np.concatenate([np.where(i1 == e)[0], np.where(i2 == e)[0]])
        wts = np.concatenate([p1[i1 == e], p2[i2 == e]])
        if len(toks) > C:
            for t_, w_ in zip(toks[C:], wts[C:]):
                overflow.append((e, int(t_), float(w_)))
            toks, wts = toks[:C], wts[:C]
        il = np.full(C, SENTINEL, dtype=np.int16)
        wl = np.zeros(C, dtype=np.float32)
        il[: len(toks)] = toks
        wl[: len(toks)] = wts
        idx_lists.append(il)
        wt_lists.append(wl)

    xb = np.zeros((T + 1, H), dtype=ml_dtypes.bfloat16)
    xb[:T] = x.astype(ml_dtypes.bfloat16)

    in_maps = []
    for e in range(E):
        # Gather + transpose this expert's tokens: xgt[p, c, s] = xb[idx_s, c*128+p]
        xg = xb[idx_lists[e].astype(np.int64)]  # [C, H]
        xgt = np.ascontiguousarray(np.transpose(xg.reshape(C, H // 128, 128), (2, 1, 0)))
        nt = (C + 127) // 128
        wt_pad = np.zeros(nt * 128, dtype=np.float32)
        wt_pad[:C] = wt_lists[e]
        wt_w = wt_pad.reshape(nt, 128).T.copy()  # [128, n_token_tiles]
        w1r = np.ascontiguousarray(
            w1[e].reshape(H // 128, 128, I // 128, 128).transpose(2, 1, 0, 3)
        )
        w3r = np.ascontiguousarray(
            w3[e].reshape(H // 128, 128, I // 128, 128).transpose(2, 1, 0, 3)
        )
        w2r = np.ascontiguousarray(
            w2[e].reshape(I // 128, 128, H // 512, 512).transpose(2, 1, 0, 3)
        )
        w1r0h = np.ascontiguousarray(w1[e][:, :128].reshape(4, 4, 128, 128).transpose(0, 2, 1, 3))
        w3r0h = np.ascontiguousarray(w3[e][:, :128].reshape(4, 4, 128, 128).transpose(0, 2, 1, 3))
        in_maps.append(
            {
                "xgt": xgt,
                "wt": wt_w,
                "w1r": w1r,
                "w3r": w3r,
                "w2r": w2r,
                "w1r0h": w1r0h,
                "w3r0h": w3r0h,
            }
        )

    nc = _get_nc()
    res = run_bass_kernel_spmd(nc, in_maps, list(range(NCORES)), trace=trace)

    out = np.zeros((T, H), dtype=np.float32)
    for e in range(E):
        ye = res.results[e]["y"]
        valid = idx_lists[e] != SENTINEL
        np.add.at(out, idx_lists[e][valid].astype(np.int64), ye[valid])
    for e, t_, w_ in overflow:
        xe = x[t_]
        g = xe @ w1[e]
        u = xe @ w3[e]
        act = (g / (1.0 + np.exp(-g))) * u
        out[t_] += w_ * (act @ w2[e])
    return out, res.exec_time_ns


def kernel(**inputs):
    out, _ = _run(inputs, trace=False)
    return out
